# revision 1
# baseline (speedup 1.0000x reference)
"""Trainium2 Bass kernel for a WaveNet-style dilated-conv stack.

Network (per reference):
  x1 = conv1d(x, Wc, bc, d=1, pad=1)                      # 1 -> 32, host-side (exact fp32)
  for l in 27 layers, d = 2^(l%9):
      g = tanh(conv(x, Wt_l, d)) * sigmoid(conv(x, Ws_l, d))   # 32->32, k=3, pad=d
      skip += conv1x1(g, Wskip_l)                              # 32->512
      x = conv1x1(g, Wdense_l) + x
  out = conv1x1(relu(conv1x1(skip, Wp1)), Wp2)            # 512->512->256
  return log_softmax(out, axis=channels)

Device strategy (8 cores, sequence-parallel):
  - Wp1 folded into skip weights on host: W1s_l = Wp1 @ Wskip_l, so
    h = Wp1@skip + bp1 is accumulated directly (512 ch), then relu -> Wp2.
  - Each core owns 16384 steps, processed as 2 halves of 8192 with a 1536-step
    halo (total receptive radius of the dilated stack is 1533).  Edge windows
    use zero/stale padding; contamination moves <= d per layer, so the valid
    region stays exact.  No cross-core communication.
  - g for 4 consecutive layers is staged in a [128, W] "ring" so the skip
    projection runs as single K=128 matmuls.  The dense 1x1 conv is a single
    K=128 matmul with zeros outside the layer's ring strip (this backend
    rejects PSUM accumulation across different PE row strips).
  - bf16 matmuls (fp32 PSUM), fp32 post-processing.
"""

import numpy as np
import ml_dtypes

BF16 = ml_dtypes.bfloat16

DIL = [2 ** i for i in range(9)] * 3
L = len(DIL)            # 27
RD, SD, QD = 32, 512, 256
T = 131072
NCORES = 8
V = T // NCORES         # 16384 per core
VH = V // 2             # 8192 per half
HALO = 1536             # >= 1533 total dilation radius
PAD = 256               # >= max dilation, so tap reads never go OOB
WH = VH + 2 * HALO      # 11264 computed window per half
WA = WH + 2 * PAD       # 11776 allocated width per half
VOFF = HALO + PAD       # 1792 valid-region offset inside the window
NGRP = (L + 3) // 4     # 7 groups of (up to) 4 layers for K=128 skip matmuls
ATILE = 1024            # activation tile width
NA = WH // ATILE        # 11 act tiles per layer per half
NB = VH // 512          # 16 valid 512-col blocks per half

_cache = {}
_last_run = {}


def _build():
    from contextlib import ExitStack

    import concourse.bacc as bacc
    import concourse.mybir as mybir
    import concourse.tile as tile

    dt = mybir.dt
    AF = mybir.ActivationFunctionType
    ALU = mybir.AluOpType
    f32, bf16 = dt.float32, dt.bfloat16

    nc = bacc.Bacc("TRN2", target_bir_lowering=False, debug=False,
                   num_devices=NCORES)

    def din(name, shape, dty):
        return nc.dram_tensor(name, shape, dty, kind="ExternalInput").ap()

    xin_d = din("xin", [RD, 2 * WA], bf16)
    wg_d = din("wg", [64, L * 3 * 64], bf16)       # gated lhsT, 2 strip replicas
    wdx_d = din("wdx", [128, L * RD], bf16)        # dense lhsT (strip-embedded)
    idw_d = din("idw", [128, 2 * RD], bf16)        # residual identity lhsT
    wskp_d = din("wskp", [128, NGRP * 4 * 128], bf16)  # skip lhsT per (grp, m)
    wp2_d = din("wp2", [128, 8 * 128], bf16)       # Wp2 lhsT per (q, p)
    bts_d = din("bts", [RD, L], f32)
    bss_d = din("bss", [RD, L], f32)
    bdc_d = din("bdc", [RD, L], f32)
    hb_d = din("hb", [128, 4], f32)
    bp2c_d = din("bp2c", [128, 2], f32)
    sumw_d = din("sumw", [128, 1], f32)
    nones_d = din("nones", [1, 128], f32)
    out_d = nc.dram_tensor("out", [QD, V], f32, kind="ExternalOutput").ap()

    with tile.TileContext(nc) as tc, ExitStack() as top:
        wp = top.enter_context(tc.tile_pool(name="wp", bufs=1))

        def load(d, tag):
            t = wp.tile(list(d.shape), d.dtype, tag=tag, name=tag)
            nc.sync.dma_start(t[:], d[:])
            return t

        wg = load(wg_d, "wg")
        wdx = load(wdx_d, "wdx")
        idw = load(idw_d, "idw")
        wskp = load(wskp_d, "wskp")
        wp2 = load(wp2_d, "wp2")
        bts = load(bts_d, "bts")
        bss = load(bss_d, "bss")
        bdc = load(bdc_d, "bdc")
        hb = load(hb_d, "hb")
        bp2c = load(bp2c_d, "bp2c")
        sumw = load(sumw_d, "sumw")
        nones = load(nones_d, "nones")

        # x ping-pongs between partition strips 0/1 of one [128, W] tensor so
        # the residual add runs on the PE as a K=128 identity matmul.
        xx = wp.tile([128, WA], bf16, tag="xx", name="xx")
        ring = wp.tile([128, WA], bf16, tag="ring", name="ring")
        h = wp.tile([128, 4 * VH], bf16, tag="h", name="h")
        nc.vector.memset(xx[:], 0.0)
        nc.vector.memset(ring[:], 0.0)

        for half in range(2):
            nc.sync.dma_start(xx[0:RD, :], xin_d[:, half * WA:(half + 1) * WA])
            with ExitStack() as lctx:
                pg = lctx.enter_context(
                    tc.tile_pool(name=f"pg{half}", bufs=3, space="PSUM"))
                pk = lctx.enter_context(
                    tc.tile_pool(name=f"pk{half}", bufs=2, space="PSUM"))
                tu = lctx.enter_context(tc.tile_pool(name=f"tu{half}", bufs=3))

                for l in range(L):
                    d = DIL[l]
                    j = l % 4
                    G = l // 4
                    sc = RD * (l % 2)        # strip of x_l
                    sn = RD * ((l + 1) % 2)  # strip of x_{l+1}
                    for a in range(NA):
                        b0 = PAD + a * ATILE
                        pgt = pg.tile([128, ATILE], f32, tag="pg", name="pg")
                        for s in range(2):
                            c0 = b0 + s * 512
                            for k in range(3):
                                nc.tensor.matmul(
                                    pgt[0:64, s * 512:(s + 1) * 512],
                                    wg[sc:sc + RD,
                                       (l * 3 + k) * 64:(l * 3 + k + 1) * 64],
                                    xx[sc:sc + RD, c0 + (k - 1) * d:
                                       c0 + (k - 1) * d + 512],
                                    start=(k == 0), stop=(k == 2),
                                    tile_position=(sc, 0))
                        tt = tu.tile([RD, ATILE], bf16, tag="t", name="t")
                        uu = tu.tile([RD, ATILE], bf16, tag="u", name="u")
                        nc.scalar.activation(tt[:], pgt[0:RD, :], AF.Tanh,
                                             bias=bts[:, l:l + 1])
                        nc.scalar.activation(uu[:], pgt[RD:2 * RD, :],
                                             AF.Sigmoid, bias=bss[:, l:l + 1])
                        nc.vector.tensor_mul(
                            ring[RD * j:RD * (j + 1), b0:b0 + ATILE],
                            tt[:], uu[:])
                        for s in range(2):
                            c0 = b0 + s * 512
                            pxs = pgt[64 + RD * s:96 + RD * s,
                                      s * 512:(s + 1) * 512]
                            nc.tensor.matmul(
                                pxs, wdx[:, l * RD:(l + 1) * RD],
                                ring[:, c0:c0 + 512], start=True, stop=False,
                                tile_position=(0, 64 + RD * s))
                            nc.tensor.matmul(
                                pxs, idw[:, RD * (l % 2):RD * (l % 2) + RD],
                                xx[:, c0:c0 + 512], start=False, stop=True,
                                tile_position=(0, 64 + RD * s))
                            # x_new = psum + bdense (residual already in psum)
                            nc.vector.tensor_scalar_add(
                                xx[sn:sn + RD, c0:c0 + 512], pxs,
                                bdc[:, l:l + 1])

                    if j == 3 or l == L - 1:
                        # skip contribution of this 4-layer group (K=128)
                        for m in range(4):
                            for cb in range(NB):
                                c0 = VOFF + cb * 512
                                pst = pk.tile([128, 512], f32, tag="pk",
                                              name="pk")
                                nc.tensor.matmul(
                                    pst[:],
                                    wskp[:, (G * 4 + m) * 128:
                                         (G * 4 + m + 1) * 128],
                                    ring[:, c0:c0 + 512],
                                    start=True, stop=True)
                                hcol = m * VH + cb * 512
                                if G == 0:
                                    nc.vector.tensor_copy(
                                        h[:, hcol:hcol + 512], pst[:])
                                else:
                                    nc.vector.tensor_add(
                                        h[:, hcol:hcol + 512],
                                        h[:, hcol:hcol + 512], pst[:])

            with ExitStack() as pctx:
                pop = pctx.enter_context(
                    tc.tile_pool(name=f"po{half}", bufs=4, space="PSUM"))
                psp = pctx.enter_context(
                    tc.tile_pool(name=f"ps{half}", bufs=2, space="PSUM"))
                pqp = pctx.enter_context(
                    tc.tile_pool(name=f"pq{half}", bufs=2, space="PSUM"))
                sp = pctx.enter_context(tc.tile_pool(name=f"sp{half}", bufs=2))
                for cb in range(NB):
                    rr = sp.tile([128, 4 * 512], bf16, tag="r", name="r")
                    for m in range(4):
                        nc.scalar.activation(
                            rr[:, m * 512:(m + 1) * 512],
                            h[:, m * VH + cb * 512:m * VH + cb * 512 + 512],
                            AF.Relu, bias=hb[:, m:m + 1])
                    pos = []
                    for p in range(2):
                        pot = pop.tile([128, 512], f32, tag="po", name="po")
                        for q in range(4):
                            nc.tensor.matmul(
                                pot[:],
                                wp2[:, (q * 2 + p) * 128:(q * 2 + p + 1) * 128],
                                rr[:, q * 512:(q + 1) * 512],
                                start=(q == 0), stop=(q == 3))
                        pos.append(pot)
                    ee = sp.tile([128, 1024], f32, tag="e", name="e")
                    for p in range(2):
                        nc.scalar.activation(ee[:, p * 512:(p + 1) * 512],
                                             pos[p][:], AF.Exp,
                                             bias=bp2c[:, p:p + 1])
                    pst = psp.tile([128, 512], f32, tag="ps", name="ps")
                    for p in range(2):
                        nc.tensor.matmul(pst[0:1, :], sumw[:],
                                         ee[:, p * 512:(p + 1) * 512],
                                         start=(p == 0), stop=(p == 1))
                    lss = sp.tile([1, 512], f32, tag="ls", name="ls")
                    nc.scalar.activation(lss[:], pst[0:1, :], AF.Ln)
                    pqt = pqp.tile([128, 512], f32, tag="pq", name="pq")
                    nc.tensor.matmul(pqt[:], nones[:], lss[:],
                                     start=True, stop=True)
                    oo = sp.tile([128, 1024], f32, tag="o", name="o")
                    oo2 = sp.tile([128, 1024], f32, tag="o2", name="o2")
                    for p in range(2):
                        nc.scalar.activation(oo[:, p * 512:(p + 1) * 512],
                                             pos[p][:], AF.Identity,
                                             bias=bp2c[:, p:p + 1])
                        nc.vector.tensor_add(oo2[:, p * 512:(p + 1) * 512],
                                             oo[:, p * 512:(p + 1) * 512],
                                             pqt[:])
                        c0 = half * VH + cb * 512
                        nc.sync.dma_start(
                            out_d[p * 128:(p + 1) * 128, c0:c0 + 512],
                            oo2[:, p * 512:(p + 1) * 512])

    nc.compile()
    return nc


def _prep_host(inputs):
    """Host-side exact fp32 preprocessing: initial conv, weight packing."""
    x = np.asarray(inputs["x"], np.float32)
    Wc = np.asarray(inputs["Wc"], np.float32)
    bc = np.asarray(inputs["bc"], np.float32)
    Wt = np.asarray(inputs["Wt"], np.float32)
    bt = np.asarray(inputs["bt"], np.float32)
    Ws = np.asarray(inputs["Ws"], np.float32)
    bs = np.asarray(inputs["bs"], np.float32)
    Wskip = np.asarray(inputs["Wskip"], np.float32)
    bskip = np.asarray(inputs["bskip"], np.float32)
    Wdense = np.asarray(inputs["Wdense"], np.float32)
    bdense = np.asarray(inputs["bdense"], np.float32)
    Wp1 = np.asarray(inputs["Wp1"], np.float32)
    bp1 = np.asarray(inputs["bp1"], np.float32)
    Wp2 = np.asarray(inputs["Wp2"], np.float32)
    bp2 = np.asarray(inputs["bp2"], np.float32)

    # initial conv (1 -> 32, k=3, pad=1), exact fp32 on host
    x0 = x[0, 0]
    xp = np.pad(x0, (1, 1))
    x1 = (Wc[:, 0, 0:1] * xp[None, 0:T]
          + Wc[:, 0, 1:2] * xp[None, 1:T + 1]
          + Wc[:, 0, 2:3] * xp[None, 2:T + 2]) + bc[:, None]
    xg = np.pad(x1, ((0, 0), (VOFF, VOFF)))

    xin = np.empty((NCORES, RD, 2 * WA), BF16)
    for c in range(NCORES):
        for hf in range(2):
            s = c * V + hf * VH
            xin[c, :, hf * WA:(hf + 1) * WA] = xg[:, s:s + WA].astype(BF16)

    wg = np.zeros((64, L * 3 * 64), np.float32)
    wdx = np.zeros((128, L * RD), np.float32)
    for l in range(L):
        for k in range(3):
            blk = np.concatenate([Wt[l, :, :, k].T, Ws[l, :, :, k].T], axis=1)
            for p in range(2):
                wg[RD * p:RD * (p + 1),
                   (l * 3 + k) * 64:(l * 3 + k + 1) * 64] = blk
        j = l % 4
        wdx[RD * j:RD * (j + 1), l * RD:(l + 1) * RD] = Wdense[l, :, :, 0].T

    idw = np.zeros((128, 2 * RD), np.float32)
    for p in range(2):
        idw[RD * p:RD * (p + 1), RD * p:RD * (p + 1)] = np.eye(RD)

    W1s = np.einsum("ab,lbc->lac", Wp1[:, :, 0], Wskip[:, :, :, 0])  # [L,512,32]
    wskp = np.zeros((128, NGRP * 4 * 128), np.float32)
    for G in range(NGRP):
        for m in range(4):
            for j in range(4):
                l = G * 4 + j
                if l < L:
                    wskp[32 * j:32 * (j + 1),
                         (G * 4 + m) * 128:(G * 4 + m + 1) * 128] = \
                        W1s[l, 128 * m:128 * (m + 1), :].T

    wp2 = np.zeros((128, 8 * 128), np.float32)
    for q in range(4):
        for p in range(2):
            wp2[:, (q * 2 + p) * 128:(q * 2 + p + 1) * 128] = \
                Wp2[128 * p:128 * (p + 1), 128 * q:128 * (q + 1), 0].T

    hbias = Wp1[:, :, 0] @ bskip.sum(axis=0) + bp1     # [512]
    hb = hbias.reshape(4, 128).T.copy()                # [128, 4]

    shared = {
        "wg": wg.astype(BF16),
        "wdx": wdx.astype(BF16),
        "idw": idw.astype(BF16),
        "wskp": wskp.astype(BF16),
        "wp2": wp2.astype(BF16),
        "bts": np.ascontiguousarray(bt.T.astype(np.float32)),
        "bss": np.ascontiguousarray(bs.T.astype(np.float32)),
        "bdc": np.ascontiguousarray(bdense.T.astype(np.float32)),
        "hb": np.ascontiguousarray(hb.astype(np.float32)),
        "bp2c": np.ascontiguousarray(bp2.reshape(2, 128).T.astype(np.float32)),
        "sumw": np.ones((128, 1), np.float32),
        "nones": np.full((1, 128), -1.0, np.float32),
    }
    return xin, shared


def kernel(**inputs):
    from concourse.bass_utils import run_bass_kernel_spmd

    xin, shared = _prep_host(inputs)
    if "nc" not in _cache:
        _cache["nc"] = _build()
    nc = _cache["nc"]

    in_maps = [dict(shared, xin=np.ascontiguousarray(xin[c]))
               for c in range(NCORES)]
    res = run_bass_kernel_spmd(nc, in_maps, core_ids=list(range(NCORES)))

    _last_run["nc"] = nc
    _last_run["in_maps"] = in_maps

    out = np.empty((1, QD, T), np.float32)
    for c in range(NCORES):
        out[0, :, c * V:(c + 1) * V] = res.results[c]["out"]
    return out



# revision 19
# speedup vs baseline: 1.6184x; 1.6184x over previous
"""Trainium2 Bass kernel for a WaveNet-style dilated-conv stack.

Network (per reference):
  x1 = conv1d(x, Wc, bc, d=1, pad=1)                      # 1 -> 32, host-side (exact fp32)
  for l in 27 layers, d = 2^(l%9):
      g = tanh(conv(x, Wt_l, d)) * sigmoid(conv(x, Ws_l, d))   # 32->32, k=3, pad=d
      skip += conv1x1(g, Wskip_l)                              # 32->512
      x = conv1x1(g, Wdense_l) + x
  out = conv1x1(relu(conv1x1(skip, Wp1)), Wp2)            # 512->512->256
  return log_softmax(out, axis=channels)

Device strategy (8 cores, sequence-parallel, no cross-core comm):
  - Wp1 folded into skip weights on host: W1s_l = Wp1 @ Wskip_l.
  - Each core owns 16384 steps as 2 halves of 8192 + halo.  Per-layer
    SHRINKING windows: layer l computes only VH + 2*sum(d_j, j>l) cols.
  - im2col along taps: x is held in 3 partition strips of a [96, W]
    buffer (x[c-d], x[c], x[c+d]) so the gated conv is ONE K=96 matmul
    per 512 cols (vs 3 taps x K=32).  The two shifted strips are
    produced by SBUF->SBUF DMA; strip roles rotate with l%3 to spread
    DMA port load across partition lines.
  - Residual x_new = dense_psum + bdense + x_old is one fused DVE
    scalar_tensor_tensor (no identity matmul on the PE).
  - g = tanh*sigmoid runs on the (otherwise idle) GpSimd engine.
  - Skip projections (K=128, 4 layers packed) accumulate 2 GROUPS per
    PSUM tile; the matmuls ride along the last layer of each odd group
    (and the post loop for the last group) so DVE h-accumulation never
    gates the PE.
  - Post: relu on DVE, exp on scalar, log-sum via matmul; Ln batched
    once per half over [1, 8192]; logits spilled to SBUF as bf16.
"""

import os

import numpy as np
import ml_dtypes

BF16 = ml_dtypes.bfloat16

MUL_ENGINE = os.environ.get("WN_MUL_ENGINE", "gpsimd")   # gpsimd | vector
REPL_MODE = os.environ.get("WN_REPL", "dma")             # dma | vector

DIL = [2 ** i for i in range(9)] * 3
L = len(DIL)            # 27
RD, SD, QD = 32, 512, 256
T = 131072
NCORES = 8
V = T // NCORES         # 16384 per core
VH = V // 2             # 8192 per half
VOFF = 2048             # valid-region offset (>= 1533 halo + margin)
WA = 12288              # allocated buffer width = VOFF + VH + VOFF
NGRP = (L + 3) // 4     # 7 groups of (up to) 4 layers

# per-layer remaining halo radius and 512-aligned compute windows
R = [sum(DIL[l + 1:]) for l in range(L)]           # g_l needed radius
LO = [((VOFF - R[l]) // 512) * 512 for l in range(L)]
HI = [-((-(VOFF + VH + R[l])) // 512) * 512 for l in range(L)]
assert all(LO[l] >= 512 and HI[l] <= WA - 512 for l in range(L))


def _check_clean():
    """Interval arithmetic: verify valid region stays exact under the
    shrinking-window schedule with edge garbage in the halo."""
    xlo, xhi = 0, WA          # clean interval of x_l values
    slo, shi = 0, WA          # clean interval of all 3 strips of B_l
    for l in range(L):
        d = DIL[l]
        glo, ghi = max(slo, LO[l]), min(shi, HI[l])   # g_l clean
        assert glo <= VOFF and ghi >= VOFF + VH, (l, glo, ghi)
        if l == L - 1:
            break
        xlo, xhi = max(glo, xlo), min(ghi, xhi)       # x_{l+1} clean
        d2 = DIL[l + 1]
        # replicas cover [LO[l]+d2, HI[l]+d2) etc; clean part:
        slo, shi = xlo + d2, xhi - d2


_check_clean()

_cache = {}
_last_run = {}


def _build():
    from contextlib import ExitStack

    import concourse.bacc as bacc
    import concourse.mybir as mybir
    import concourse.tile as tile

    dt = mybir.dt
    AF = mybir.ActivationFunctionType
    ALU = mybir.AluOpType
    f32, bf16 = dt.float32, dt.bfloat16

    nc = bacc.Bacc("TRN2", target_bir_lowering=False, debug=False,
                   num_devices=NCORES)

    def din(name, shape, dty):
        return nc.dram_tensor(name, shape, dty, kind="ExternalInput").ap()

    xin_d = din("xin", [96, 2 * WA], bf16)
    wg_d = din("wg", [96, L * 64], bf16)          # gated lhsT per layer
    wdn_d = din("wdn", [128, L * 32], bf16)       # dense lhsT (strip l%4)
    wskp_d = din("wskp", [128, NGRP * 4 * 128], bf16)  # skip lhsT per (grp, m)
    wp2_d = din("wp2", [128, 8 * 128], bf16)      # Wp2 lhsT per (q, p)
    bts_d = din("bts", [RD, L], f32)
    bss_d = din("bss", [RD, L], f32)
    bdc_d = din("bdc", [96, L], f32)   # bdense replicated across 3 strips
    hb_d = din("hb", [128, 4], f32)
    bp2c_d = din("bp2c", [128, 2], f32)
    sumw_d = din("sumw", [128, 128], bf16)
    out_d = nc.dram_tensor("out", [QD, V], f32, kind="ExternalOutput").ap()

    with tile.TileContext(nc) as tc, ExitStack() as top:
        wp = top.enter_context(tc.tile_pool(name="wp", bufs=1))

        def load(d, tag):
            t = wp.tile(list(d.shape), d.dtype, tag=tag, name=tag)
            nc.sync.dma_start(t[:], d[:])
            return t

        wg = load(wg_d, "wg")
        wdn = load(wdn_d, "wdn")
        wskp = load(wskp_d, "wskp")
        wp2 = load(wp2_d, "wp2")
        bts = load(bts_d, "bts")
        bss = load(bss_d, "bss")
        bdc = load(bdc_d, "bdc")
        hb = load(hb_d, "hb")
        bp2c = load(bp2c_d, "bp2c")
        sumw = load(sumw_d, "sumw")

        bb = [wp.tile([96, WA], bf16, tag=f"b{i}", name=f"b{i}")
              for i in range(2)]
        ring = [wp.tile([128, WA], bf16, tag=f"r{i}", name=f"r{i}")
                for i in range(2)]
        h = wp.tile([128, 4 * VH], bf16, tag="h", name="h")
        mule = nc.gpsimd if MUL_ENGINE == "gpsimd" else nc.vector
        nc.vector.memset(bb[1][:], 0.0)
        mule.memset(ring[0][:], 0.0)
        mule.memset(ring[1][:], 0.0)

        for half in range(2):
            nc.sync.dma_start(bb[0][:], xin_d[:, half * WA:(half + 1) * WA])

            with ExitStack() as lctx:
                pg = lctx.enter_context(
                    tc.tile_pool(name=f"pg{half}", bufs=3, space="PSUM"))
                pk = lctx.enter_context(
                    tc.tile_pool(name=f"pk{half}", bufs=2, space="PSUM"))
                tu = lctx.enter_context(tc.tile_pool(name=f"tu{half}", bufs=3))

                def emit_skip_pair(Gs, cb, rg):
                    """skip matmuls for groups Gs on valid block cb, psum-
                    accumulated, then one DVE h update per m strip."""
                    c0 = VOFF + cb * 512
                    for m in range(4):
                        pst = pk.tile([128, 512], f32, tag="pk", name="pk")
                        for gi, G in enumerate(Gs):
                            nc.tensor.matmul(
                                pst[:],
                                wskp[:, (G * 4 + m) * 128:
                                     (G * 4 + m + 1) * 128],
                                ring[G % 2][:, c0:c0 + 512],
                                start=(gi == 0), stop=(gi == len(Gs) - 1))
                        hcol = m * VH + cb * 512
                        if Gs[0] == 0:
                            # first contribution: h = pst + hbias
                            nc.vector.tensor_scalar_add(
                                h[:, hcol:hcol + 512], pst[:],
                                hb[:, m:m + 1])
                        else:
                            nc.vector.tensor_add(
                                h[:, hcol:hcol + 512],
                                h[:, hcol:hcol + 512], pst[:])

                for l in range(L):
                    d = DIL[l]
                    j = l % 4
                    G = l // 4
                    rg = ring[G % 2]
                    r_in = l % 3          # center strip of B_l
                    bin_, bout = bb[l % 2], bb[(l + 1) % 2]
                    lo, hi = LO[l], HI[l]
                    # strip order -> tap offsets for this layer's weights
                    # (weights packed host-side to match)
                    last = l % 8 == 7
                    pend = list(range(16)) if last else []

                    c = lo
                    while c < hi:
                        cw = min(1024, hi - c)
                        pgt = pg.tile([128, 1024], f32, tag="pg", name="pg")
                        for s in range(0, cw, 512):
                            nc.tensor.matmul(
                                pgt[0:64, s:s + 512],
                                wg[:, l * 64:(l + 1) * 64],
                                bin_[0:96, c + s:c + s + 512],
                                start=True, stop=True, tile_position=(0, 0))
                        tt = tu.tile([RD, 1024], bf16, tag="t", name="t")
                        uu = tu.tile([RD, 1024], bf16, tag="u", name="u")
                        nc.scalar.activation(tt[:, 0:cw], pgt[0:RD, 0:cw],
                                             AF.Tanh, bias=bts[:, l:l + 1])
                        nc.scalar.activation(uu[:, 0:cw], pgt[RD:64, 0:cw],
                                             AF.Sigmoid, bias=bss[:, l:l + 1])
                        mule.tensor_mul(
                            rg[RD * j:RD * (j + 1), c:c + cw],
                            tt[:, 0:cw], uu[:, 0:cw])
                        if l < L - 1:
                            d2 = DIL[l + 1]
                            r_out = (l + 1) % 3
                            for s in range(0, cw, 512):
                                nc.tensor.matmul(
                                    pgt[64:96, s:s + 512],
                                    wdn[RD * j:RD * (j + 1),
                                        l * 32:(l + 1) * 32],
                                    rg[RD * j:RD * (j + 1),
                                       c + s:c + s + 512],
                                    start=True, stop=True,
                                    tile_position=(RD * j, 64))
                            # x_new = dense + bdense + x_old (fused)
                            nc.vector.scalar_tensor_tensor(
                                bout[RD * r_out:RD * (r_out + 1), c:c + cw],
                                pgt[64:96, 0:cw],
                                bdc[RD * r_in:RD * (r_in + 1), l:l + 1],
                                bin_[RD * r_in:RD * (r_in + 1), c:c + cw],
                                ALU.add, ALU.add)
                            # shifted tap replicas for the next layer
                            rl = (r_out + 1) % 3      # holds x[c-d2]
                            rr_ = (r_out + 2) % 3     # holds x[c+d2]
                            src = bout[RD * r_out:RD * (r_out + 1), c:c + cw]
                            if REPL_MODE == "dma":
                                nc.sync.dma_start(
                                    bout[RD * rl:RD * (rl + 1),
                                         c + d2:c + cw + d2], src)
                                nc.sync.dma_start(
                                    bout[RD * rr_:RD * (rr_ + 1),
                                         c - d2:c + cw - d2], src)
                            else:
                                nc.vector.tensor_copy(
                                    bout[RD * rl:RD * (rl + 1),
                                         c + d2:c + cw + d2], src)
                                nc.vector.tensor_copy(
                                    bout[RD * rr_:RD * (rr_ + 1),
                                         c - d2:c + cw - d2], src)
                        # interleave pending paired-group skip work
                        while pend and VOFF + (pend[0] + 1) * 512 <= c + cw:
                            emit_skip_pair((G - 1, G), pend.pop(0), rg)
                        c += cw
                    for cb in pend:
                        emit_skip_pair((G - 1, G), cb, rg)

            with ExitStack() as pctx:
                pop = pctx.enter_context(
                    tc.tile_pool(name=f"po{half}", bufs=2, space="PSUM"))
                pk6 = pctx.enter_context(
                    tc.tile_pool(name=f"pk6{half}", bufs=2, space="PSUM"))
                psp = pctx.enter_context(
                    tc.tile_pool(name=f"ps{half}", bufs=2, space="PSUM"))
                sp = pctx.enter_context(tc.tile_pool(name=f"sp{half}", bufs=2))

                for cb in range(16):
                    c0 = VOFF + cb * 512
                    # last skip group rides along with the post loop
                    for m in range(4):
                        pst = pk6.tile([128, 512], f32, tag="pk6", name="pk6")
                        nc.tensor.matmul(
                            pst[:],
                            wskp[:, (24 + m) * 128:(24 + m + 1) * 128],
                            ring[0][:, c0:c0 + 512], start=True, stop=True)
                        hcol = m * VH + cb * 512
                        nc.vector.tensor_add(
                            h[:, hcol:hcol + 512],
                            h[:, hcol:hcol + 512], pst[:])
                    rr = sp.tile([128, 4 * 512], bf16, tag="rr", name="rr")
                    for m in range(4):
                        hcol = m * VH + cb * 512
                        if m < 2:
                            nc.scalar.activation(
                                rr[:, m * 512:(m + 1) * 512],
                                h[:, hcol:hcol + 512], AF.Relu)
                        else:
                            nc.vector.tensor_scalar_max(
                                rr[:, m * 512:(m + 1) * 512],
                                h[:, hcol:hcol + 512], 0.0)
                    pot = pop.tile([128, 1024], f32, tag="po", name="po")
                    for p in range(2):
                        for q in range(4):
                            nc.tensor.matmul(
                                pot[:, p * 512:(p + 1) * 512],
                                wp2[:, (q * 2 + p) * 128:(q * 2 + p + 1) * 128],
                                rr[:, q * 512:(q + 1) * 512],
                                start=(q == 0), stop=(q == 3))
                    ee = sp.tile([128, 1024], bf16, tag="ee", name="ee")
                    for p in range(2):
                        nc.scalar.activation(ee[:, p * 512:(p + 1) * 512],
                                             pot[:, p * 512:(p + 1) * 512],
                                             AF.Exp, bias=bp2c[:, p:p + 1])
                    # sum over all 256 channels, broadcast to 128 partitions
                    pst = psp.tile([128, 512], f32, tag="ps", name="ps")
                    for p in range(2):
                        nc.tensor.matmul(pst[:], sumw[:],
                                         ee[:, p * 512:(p + 1) * 512],
                                         start=(p == 0), stop=(p == 1))
                    lnb = sp.tile([128, 512], f32, tag="lnb", name="lnb")
                    nc.scalar.activation(lnb[:], pst[:], AF.Ln)
                    oo = sp.tile([128, 1024], f32, tag="oo", name="oo")
                    for p in range(2):
                        nc.vector.scalar_tensor_tensor(
                            oo[:, p * 512:(p + 1) * 512],
                            pot[:, p * 512:(p + 1) * 512],
                            bp2c[:, p:p + 1], lnb[:],
                            ALU.add, ALU.subtract)
                        c0 = half * VH + cb * 512
                        nc.sync.dma_start(
                            out_d[p * 128:(p + 1) * 128, c0:c0 + 512],
                            oo[:, p * 512:(p + 1) * 512])

    nc.compile()
    return nc


def _prep_host(inputs):
    """Host-side exact fp32 preprocessing: initial conv, weight packing."""
    x = np.asarray(inputs["x"], np.float32)
    Wc = np.asarray(inputs["Wc"], np.float32)
    bc = np.asarray(inputs["bc"], np.float32)
    Wt = np.asarray(inputs["Wt"], np.float32)
    bt = np.asarray(inputs["bt"], np.float32)
    Ws = np.asarray(inputs["Ws"], np.float32)
    bs = np.asarray(inputs["bs"], np.float32)
    Wskip = np.asarray(inputs["Wskip"], np.float32)
    bskip = np.asarray(inputs["bskip"], np.float32)
    Wdense = np.asarray(inputs["Wdense"], np.float32)
    bdense = np.asarray(inputs["bdense"], np.float32)
    Wp1 = np.asarray(inputs["Wp1"], np.float32)
    bp1 = np.asarray(inputs["bp1"], np.float32)
    Wp2 = np.asarray(inputs["Wp2"], np.float32)
    bp2 = np.asarray(inputs["bp2"], np.float32)

    # initial conv (1 -> 32, k=3, pad=1), exact fp32 on host
    x0 = x[0, 0]
    xp = np.pad(x0, (1, 1))
    x1 = (Wc[:, 0, 0:1] * xp[None, 0:T]
          + Wc[:, 0, 1:2] * xp[None, 1:T + 1]
          + Wc[:, 0, 2:3] * xp[None, 2:T + 2]) + bc[:, None]
    xg = np.pad(x1, ((0, 0), (VOFF + 8, VOFF + 8)))

    # layer-0 strip roles: center=0, left(x[c-1])=1, right(x[c+1])=2
    xin = np.empty((NCORES, 96, 2 * WA), BF16)
    for c in range(NCORES):
        for hf in range(2):
            s = c * V + hf * VH + 8
            xin[c, 0:32, hf * WA:(hf + 1) * WA] = xg[:, s:s + WA]
            xin[c, 32:64, hf * WA:(hf + 1) * WA] = xg[:, s - 1:s - 1 + WA]
            xin[c, 64:96, hf * WA:(hf + 1) * WA] = xg[:, s + 1:s + 1 + WA]

    # gated weights: rows 32s.. = tap seen by strip s of layer l
    #   strip r=l%3 -> center tap (k=1), (r+1)%3 -> left (k=0),
    #   (r+2)%3 -> right (k=2)
    wg = np.zeros((96, L * 64), np.float32)
    for l in range(L):
        r = l % 3
        kof = {r: 1, (r + 1) % 3: 0, (r + 2) % 3: 2}
        for s3 in range(3):
            k = kof[s3]
            blk = np.concatenate([Wt[l, :, :, k].T, Ws[l, :, :, k].T], axis=1)
            wg[32 * s3:32 * (s3 + 1), l * 64:(l + 1) * 64] = blk

    wdn = np.zeros((128, L * 32), np.float32)
    for l in range(L):
        j = l % 4
        wdn[32 * j:32 * (j + 1), l * 32:(l + 1) * 32] = Wdense[l, :, :, 0].T

    W1s = np.einsum("ab,lbc->lac", Wp1[:, :, 0], Wskip[:, :, :, 0])  # [L,512,32]
    wskp = np.zeros((128, NGRP * 4 * 128), np.float32)
    for G in range(NGRP):
        for m in range(4):
            for j in range(4):
                l = G * 4 + j
                if l < L:
                    wskp[32 * j:32 * (j + 1),
                         (G * 4 + m) * 128:(G * 4 + m + 1) * 128] = \
                        W1s[l, 128 * m:128 * (m + 1), :].T

    wp2 = np.zeros((128, 8 * 128), np.float32)
    for q in range(4):
        for p in range(2):
            wp2[:, (q * 2 + p) * 128:(q * 2 + p + 1) * 128] = \
                Wp2[128 * p:128 * (p + 1), 128 * q:128 * (q + 1), 0].T

    hbias = Wp1[:, :, 0] @ bskip.sum(axis=0) + bp1     # [512]
    hb = hbias.reshape(4, 128).T.copy()                # [128, 4]

    shared = {
        "wg": wg.astype(BF16),
        "wdn": wdn.astype(BF16),
        "wskp": wskp.astype(BF16),
        "wp2": wp2.astype(BF16),
        "bts": np.ascontiguousarray(bt.T.astype(np.float32)),
        "bss": np.ascontiguousarray(bs.T.astype(np.float32)),
        "bdc": np.ascontiguousarray(
            np.tile(bdense.T, (3, 1)).astype(np.float32)),
        "hb": np.ascontiguousarray(hb.astype(np.float32)),
        "bp2c": np.ascontiguousarray(bp2.reshape(2, 128).T.astype(np.float32)),
        "sumw": np.ones((128, 128), BF16),
    }
    return xin, shared


def kernel(**inputs):
    from concourse.bass_utils import run_bass_kernel_spmd

    xin, shared = _prep_host(inputs)
    if "nc" not in _cache:
        _cache["nc"] = _build()
    nc = _cache["nc"]

    in_maps = [dict(shared, xin=np.ascontiguousarray(xin[c]))
               for c in range(NCORES)]
    res = run_bass_kernel_spmd(nc, in_maps, core_ids=list(range(NCORES)))

    _last_run["nc"] = nc
    _last_run["in_maps"] = in_maps

    out = np.empty((1, QD, T), np.float32)
    for c in range(NCORES):
        out[0, :, c * V:(c + 1) * V] = res.results[c]["out"]
    return out
